# revision 9
# baseline (speedup 1.0000x reference)
"""GRU (ClassicRNN over ragged sequences) Trainium2 Bass kernel.

Full-input contract: kernel(**inputs) takes the unsharded numpy inputs
(B=256, T=512, V=64, H=256) and returns the full [256, 256] h_final.
Internally the batch is sharded 8 ways (data parallel; GRU weights
replicated), one shard per NeuronCore, run SPMD via run_bass_kernel_spmd.

Math notes:
  x_t = concat(data_t, mask_t, delta_t)            # [B, 129]
  gx  = x_t @ W_ih.T + b_ih                        # precomputed for all t
  gh  = h @ W_hh.T (+ b_hh)                        # per step
  r = sig(gxr + ghr + b_hhr); z = sig(gxz + ghz + b_hhz)
  n = tanh(gxn + r * (ghn + b_hhn))
  h = (1-z) n + z h, carried only where t_exist
Ragged masking trick: where t_exist is False we add +40 to the z-gate
preactivation during precompute, so z saturates to exactly 1.0 in fp32 and
h carries through unchanged. This removes masking from the recurrent loop.

Layouts (per core):
  Gate-major "arr" layout: [128 part, 6 gate-tiles, 32 batch]; gate row
  g*128+p of the 768 gate rows lives at partition p, tile g. r=tiles 0..1,
  z=2..3, n=4..5. Hidden state h is kept transposed as [128, 2, 32]
  (tiles 0..1 of the same indexing), which is exactly the moving operand
  the recurrent matmul needs -> no per-step transposes.
"""

import os
import sys

import numpy as np

sys.path.insert(0, "/opt/trn_rl_repo")

B_FULL = 256
NCORES = 8
B = B_FULL // NCORES  # 32 per-core batch
T = 512
V = 64
H = 256
G = 3 * H  # 768 gate rows
NG = G // 128  # 6 gate tiles
KK = H // 128  # 2 hidden k-tiles
F = 2 * V  # 128 concat(data, mask) features
GB = NG * B  # 192 free elems per step in arr layout
TC = 16  # timesteps per phase-A chunk (16*32 = 512 matmul columns)
NCHUNK = T // TC
ZBIG = 40.0  # z-gate preactivation offset for masked steps

_CACHE = {}


def _build(t_steps=T, recur_dtype="float32"):
    import concourse.bass as bass
    import concourse.tile as tile
    from concourse import bacc, mybir
    from concourse.masks import make_identity

    fp32 = mybir.dt.float32
    AF = mybir.ActivationFunctionType
    OP = mybir.AluOpType

    nc = bacc.Bacc("TRN2", target_bir_lowering=False, debug=False, num_devices=NCORES)

    times_d = nc.dram_tensor("times_in", [B, T], fp32, kind="ExternalInput").ap()
    data_d = nc.dram_tensor("data_in", [B, T, V], fp32, kind="ExternalInput").ap()
    mask_d = nc.dram_tensor("mask_in", [B, T, V], fp32, kind="ExternalInput").ap()
    wih_d = nc.dram_tensor("W_ih", [G, 2 * V + 1], fp32, kind="ExternalInput").ap()
    whh_d = nc.dram_tensor("W_hh", [G, H], fp32, kind="ExternalInput").ap()
    bih_d = nc.dram_tensor("b_ih", [G], fp32, kind="ExternalInput").ap()
    bhh_d = nc.dram_tensor("b_hh", [G], fp32, kind="ExternalInput").ap()
    hout_d = nc.dram_tensor("h_out", [B, H], fp32, kind="ExternalOutput").ap()
    gx_d = nc.dram_tensor("gx_scratch", [T, 128, GB], fp32, kind="Internal").ap()

    nchunk = t_steps // TC

    with tile.TileContext(nc) as tc:
        with (
            tc.tile_pool(name="const", bufs=1) as const,
            tc.tile_pool(name="wload", bufs=2) as wload,
            tc.tile_pool(name="ptr", bufs=2, space=bass.MemorySpace.PSUM) as ptr,
        ):
            ident = const.tile([128, 128], fp32)
            make_identity(nc, ident)

            # ---- weights into transposed sbuf layouts ----
            # WcatT[f, r] = W_ih[r, f] for f in 0..127 (data+mask features)
            wcatT = const.tile([128, G], fp32)
            for g in range(NG):
                wt = wload.tile([128, 2 * V + 1], fp32, tag="wih")
                nc.sync.dma_start(out=wt, in_=wih_d[g * 128 : (g + 1) * 128, :])
                ps = ptr.tile([128, 128], fp32, tag="tr")
                nc.tensor.transpose(ps, wt[:, 0:F], ident)
                nc.vector.tensor_copy(out=wcatT[:, g * 128 : (g + 1) * 128], in_=ps)

            # WhhT[k, kk, r] = W_hh[r, kk*128 + k]
            whhT = const.tile([128, KK, G], fp32)
            for g in range(NG):
                wt = wload.tile([128, H], fp32, tag="whh")
                nc.sync.dma_start(out=wt, in_=whh_d[g * 128 : (g + 1) * 128, :])
                for kk in range(KK):
                    ps = ptr.tile([128, 128], fp32, tag="tr")
                    nc.tensor.transpose(ps, wt[:, kk * 128 : (kk + 1) * 128], ident)
                    nc.vector.tensor_copy(
                        out=whhT[:, kk, g * 128 : (g + 1) * 128], in_=ps
                    )

            # aux_w rows: [b_fold | w_dt | e_z] per gate-row column.
            # Engine ops can't start at partition>0, so build rows in
            # partition-0 temps and DMA them into place.
            aux_w = const.tile([3, G], fp32)
            bias_row = wload.tile([1, G], fp32, tag="bias_row")
            nc.sync.dma_start(out=bias_row, in_=bih_d[None, :])
            bhh_row = wload.tile([1, G], fp32, tag="bhh_row")
            nc.sync.dma_start(out=bhh_row, in_=bhh_d[None, :])
            nc.vector.tensor_add(
                out=bias_row[:, 0:512], in0=bias_row[:, 0:512], in1=bhh_row[:, 0:512]
            )
            nc.sync.dma_start(out=aux_w[0:1, :], in_=bias_row)
            nc.gpsimd.dma_start(
                out=aux_w[1:2, :], in_=wih_d[:, F : F + 1].rearrange("a b -> b a")
            )
            ez_row = wload.tile([1, G], fp32, tag="ez_row")
            nc.vector.memset(ez_row, 0.0)
            nc.vector.memset(ez_row[:, 256:512], 1.0)
            nc.sync.dma_start(out=aux_w[2:3, :], in_=ez_row)

            # b_hh n-gate columns [128, KK]
            bhh_n = const.tile([128, KK], fp32)
            for kk in range(KK):
                nc.sync.dma_start(
                    out=bhh_n[:, kk : kk + 1],
                    in_=bhh_d[512 + kk * 128 : 512 + (kk + 1) * 128][:, None],
                )

            # ---- times -> deltaT, cT in t-major transposed layout ----
            times_p = const.tile([B, 520], fp32)
            nc.vector.memset(times_p, 0.0)
            nc.sync.dma_start(out=times_p[:, 0:T], in_=times_d)
            # so delta[:, T-1] = 0: pad col T repeats col T-1
            nc.vector.tensor_copy(
                out=times_p[:, T : T + 1], in_=times_p[:, T - 1 : T]
            )
            timesT = const.tile([128, 4, B], fp32)
            timesTs = const.tile([128, 4, B], fp32)
            for q in range(4):
                ps = ptr.tile([128, B], fp32, tag="tr")
                nc.tensor.transpose(
                    ps, times_p[:, q * 128 : (q + 1) * 128], ident[0:B, 0:B]
                )
                nc.vector.tensor_copy(out=timesT[:, q, :], in_=ps)
                ps = ptr.tile([128, B], fp32, tag="tr")
                nc.tensor.transpose(
                    ps, times_p[:, q * 128 + 1 : (q + 1) * 128 + 1], ident[0:B, 0:B]
                )
                nc.vector.tensor_copy(out=timesTs[:, q, :], in_=ps)
            deltaT = const.tile([128, 4, B], fp32)
            nc.vector.tensor_sub(out=deltaT, in0=timesTs, in1=timesT)
            cT = const.tile([128, 4, B], fp32)
            nc.vector.tensor_scalar(
                out=cT, in0=timesT, scalar1=0.0, scalar2=ZBIG, op0=OP.is_le, op1=OP.mult
            )

            # aux_rows [3, T*B] in (t, b) column order: ones / delta / c
            # (pairs with aux_w rows b_fold / w_dt / e_z)
            aux_rows = const.tile([3, T * B], fp32)
            nc.vector.memset(aux_rows[0:1, :], 1.0)
            for q in range(4):
                nc.sync.dma_start(
                    out=aux_rows[1:2, q * 4096 : (q + 1) * 4096].rearrange(
                        "a (p b) -> a p b", b=B
                    ),
                    in_=deltaT[:, q, :],
                )
                nc.sync.dma_start(
                    out=aux_rows[2:3, q * 4096 : (q + 1) * 4096].rearrange(
                        "a (p b) -> a p b", b=B
                    ),
                    in_=cT[:, q, :],
                )

            # persistent hidden state, transposed arr layout
            h_arr = const.tile([128, KK, B], fp32)
            nc.vector.memset(h_arr, 0.0)

            # ================= Phase A: gx precompute =================
            data_t = data_d.rearrange("b t v -> t b v")
            mask_t = mask_d.rearrange("b t v -> t b v")
            with (
                tc.tile_pool(name="ax", bufs=3) as ax,
                tc.tile_pool(name="agx", bufs=2) as agx,
                tc.tile_pool(name="aps", bufs=2, space=bass.MemorySpace.PSUM) as aps,
                tc.tile_pool(name="apg", bufs=4, space=bass.MemorySpace.PSUM) as apg,
            ):
                for c in range(nchunk):
                    xcatT = ax.tile([128, 512], fp32, tag="xcatT")
                    for q in range(4):
                        t0 = c * TC + q * 4
                        blk = ax.tile([128, F], fp32, tag="blk")
                        nc.sync.dma_start(
                            out=blk[:, 0:V], in_=data_t[t0 : t0 + 4]
                        )
                        nc.sync.dma_start(
                            out=blk[:, V:F], in_=mask_t[t0 : t0 + 4]
                        )
                        ps = aps.tile([128, 128], fp32, tag="xtr")
                        nc.tensor.transpose(ps, blk, ident)
                        nc.vector.tensor_copy(
                            out=xcatT[:, q * 128 : (q + 1) * 128], in_=ps
                        )
                    gx_sb = agx.tile([128, TC, NG, B], fp32, tag="gx_sb")
                    for g in range(NG):
                        pg = apg.tile([128, 512], fp32, tag="pg")
                        nc.tensor.matmul(
                            pg,
                            wcatT[:, g * 128 : (g + 1) * 128],
                            xcatT,
                            start=True,
                            stop=False,
                        )
                        nc.tensor.matmul(
                            pg,
                            aux_w[:, g * 128 : (g + 1) * 128],
                            aux_rows[:, c * 512 : (c + 1) * 512],
                            start=False,
                            stop=True,
                        )
                        nc.vector.tensor_copy(
                            out=gx_sb[:, :, g, :],
                            in_=pg.rearrange("p (t b) -> p t b", b=B),
                        )
                    nc.scalar.dma_start(
                        out=gx_d[c * TC : (c + 1) * TC].rearrange("t p f -> p t f"),
                        in_=gx_sb.rearrange("p t g b -> p t (g b)"),
                    )

            # ================= Phase B: recurrence =================
            with (
                tc.tile_pool(name="bgx", bufs=8) as bgx,
                tc.tile_pool(name="bwork", bufs=3) as bwork,
                tc.tile_pool(name="bps", bufs=4, space=bass.MemorySpace.PSUM) as bps,
            ):
                for t in range(t_steps):
                    gxt = bgx.tile([128, NG, B], fp32, tag="gxt")
                    nc.sync.dma_start(
                        out=gxt, in_=gx_d[t].rearrange("p (g b) -> p g b", b=B)
                    )
                    pss = bps.tile([128, NG, B], fp32, tag="pss")
                    for g in range(NG):
                        for kk in range(KK):
                            nc.tensor.matmul(
                                pss[:, g, :],
                                whhT[:, kk, g * 128 : (g + 1) * 128],
                                h_arr[:, kk, :],
                                start=(kk == 0),
                                stop=(kk == KK - 1),
                            )
                    srz = bwork.tile([128, 4, B], fp32, tag="srz")
                    nc.vector.tensor_add(out=srz, in0=gxt[:, 0:4, :], in1=pss[:, 0:4, :])
                    rz = bwork.tile([128, 4, B], fp32, tag="rz")
                    nc.scalar.activation(out=rz, in_=srz, func=AF.Sigmoid)
                    rhn = bwork.tile([128, KK, B], fp32, tag="rhn")
                    for j in range(KK):
                        nc.vector.scalar_tensor_tensor(
                            out=rhn[:, j, :],
                            in0=pss[:, 4 + j, :],
                            scalar=bhh_n[:, j : j + 1],
                            in1=rz[:, j, :],
                            op0=OP.add,
                            op1=OP.mult,
                        )
                    sn = bwork.tile([128, KK, B], fp32, tag="sn")
                    nc.vector.tensor_add(out=sn, in0=gxt[:, 4:6, :], in1=rhn)
                    n_t = bwork.tile([128, KK, B], fp32, tag="n_t")
                    nc.scalar.activation(out=n_t, in_=sn, func=AF.Tanh)
                    hmn = bwork.tile([128, KK, B], fp32, tag="hmn")
                    nc.vector.tensor_sub(out=hmn, in0=h_arr, in1=n_t)
                    zh = bwork.tile([128, KK, B], fp32, tag="zh")
                    nc.vector.tensor_mul(out=zh, in0=rz[:, 2:4, :], in1=hmn)
                    nc.vector.tensor_add(out=h_arr, in0=n_t, in1=zh)

            # ================= Phase C: output =================
            with tc.tile_pool(name="outp", bufs=2) as outp:
                h_row = outp.tile([B, H], fp32)
                for kk in range(KK):
                    ps = ptr.tile([128, 128], fp32, tag="tr")
                    nc.tensor.transpose(ps[0:B, :], h_arr[:, kk, :], ident)
                    nc.vector.tensor_copy(
                        out=h_row[:, kk * 128 : (kk + 1) * 128], in_=ps[0:B, :]
                    )
                nc.sync.dma_start(out=hout_d, in_=h_row)

    nc.compile()
    return nc


def _get_nc(t_steps=T):
    key = ("nc", t_steps)
    if key not in _CACHE:
        _CACHE[key] = _build(t_steps)
    return _CACHE[key]


def kernel(times_in, data_in, mask_in, W_ih, W_hh, b_ih, b_hh, trace=False):
    from concourse import bass_utils

    nc = _get_nc()
    in_maps = []
    for i in range(NCORES):
        sl = slice(i * B, (i + 1) * B)
        in_maps.append(
            {
                "times_in": np.ascontiguousarray(times_in[sl]).astype(np.float32),
                "data_in": np.ascontiguousarray(data_in[sl]).astype(np.float32),
                "mask_in": np.ascontiguousarray(mask_in[sl]).astype(np.float32),
                "W_ih": np.asarray(W_ih, np.float32),
                "W_hh": np.asarray(W_hh, np.float32),
                "b_ih": np.asarray(b_ih, np.float32),
                "b_hh": np.asarray(b_hh, np.float32),
            }
        )
    res = bass_utils.run_bass_kernel_spmd(
        nc, in_maps, core_ids=list(range(NCORES)), trace=trace
    )
    out = np.concatenate([res.results[i]["h_out"] for i in range(NCORES)], axis=0)
    if trace:
        _CACHE["last_exec_time_ns"] = res.exec_time_ns
    return out
